# revision 27
# baseline (speedup 1.0000x reference)
"""TRN2 Bass kernel for nn_AttributeClassifierHeaders (dense per-head MLP).

Computes y[b, a] = sigmoid(gelu(x @ W1[a] + b1[a]) . W2[a] + b2[a]) for 40
heads, sharded 5 heads per NeuronCore across 8 cores (head-parallel: each
head's weights are independent; x is replicated).

Stage-1 (the 2048x1024 per-head GEMM, 97% of FLOPs) runs on the PE in
fp8-e4m3 with perf_mode=DoubleRow: two fp8 weights per PE cell contract
K=256 per matmul at ~2x the fp32r/bf16 MAC rate (hw-measured; the fp32r
baseline at 1542us dropped to ~650-700us). x is quantized host-side to
e4m3*2^5 and W1 to e4m3*2^13; the 2^-18 descale rides the gelu's scale
operand on ScalarE (ht = gelu(psum*2^-18 + b1), bf16 out). Measured
end-to-end rel err vs the fp32 reference: 1.33e-2 (tolerance 2e-2, and
exactly the host-side fp8 quantization error -- the hw pipeline adds
nothing).

ScalarE is the near-critical engine (~1300 gelu instructions nearly
match the PE's busy time), so chunks are processed in PAIRS: two
512-batch chunks of the same m-tile share the b1 bias column, letting
one ACT gelu (and one DVE op) cover a [128, 1024] region spanning two
PSUM banks -- halving ScalarE/VectorE instruction count and their fixed
overheads. Stage-2 (the per-head dot over hid) runs as a serial
multiply-accumulate on the otherwise-idle DVE (scalar_tensor_tensor:
acc += ht * w2[m], w2 column as the per-partition scalar), finished by
one ones-stationary bf16 matmul per 512-batch chunk that reduces the
128 partitions into the logit row. Those tail matmuls are batch-flushed
once per job (one DR->bf16 PE mode-switch site per job, with >=1 chunk
of deferral) so the in-order PE queue never waits on the ACT->DVE
chain. Stage-2 must NOT be
f32r: the PE pulls LDWEIGHTS ahead of in-flight matmuls, and an fp8-DR
LDWEIGHTS hoisted into an f32r matmul's internal two-pass 4-byte weight
load corrupts it (hw-bisected: stage-1 h dumps were bit-exact while f32r
stage-2 logits were wrong). bf16/fp8 weight loads are immune.

x is resident as two double-buffered batch halves (W1 re-streamed per
half, 21 MiB/iter total -- trivial next to compute) so x DMA overlaps
compute across iteration boundaries and startup waits on only half the
batch. W1 heads are prefetched two jobs (~110us) ahead of use.

Layouts (host-packed):
  x8[kb, p, i, b]        = e4m3(x[b, kb*256 + i*128 + p] * 2^5)   moving
  w18[a, m, p, kb, i, c] = e4m3(W1[a, kb*256+i*128+p, m*128+c] * 2^13)
so each DoubleRow matmul takes stationary [128, 2, 128] (i-stride 128 B)
and moving [128, 2, 512] (i-stride bh bytes), matching the hw's
[Ki, Ko=2, dim] access-pattern requirement (step % 16 == 0; pairing
verified on hw by probe_dr.py/probe_dr2.py).
"""
import os
import sys
from contextlib import ExitStack

import numpy as np
import ml_dtypes

for _p in ("/root/.axon_site/_ro/trn_rl_repo", "/opt/trn_rl_repo"):
    if os.path.isdir(_p) and _p not in sys.path:
        sys.path.append(_p)

import jax  # noqa: E402
from jax.sharding import Mesh, PartitionSpec, NamedSharding  # noqa: E402
from jax.experimental.shard_map import shard_map  # noqa: E402

import concourse.bacc as bacc  # noqa: E402
import concourse.tile as tile  # noqa: E402
from concourse import mybir, bass2jax  # noqa: E402

F32 = mybir.dt.float32
F32R = mybir.dt.float32r
BF16 = mybir.dt.bfloat16
F8 = mybir.dt.float8e4
AF = mybir.ActivationFunctionType
DR = mybir.MatmulPerfMode.DoubleRow

# problem shape (hardcoded; see module docstring)
B, D, A, H = 4096, 2048, 40, 1024
NCORES = 8
APC = A // NCORES        # 5 heads per core
KB = D // 256            # 8 contraction blocks of 256 (DoubleRow)
MT = H // 128            # 8 hid tiles
XS = 5                   # x quant scale exponent  (x * 2^5)
WS = 13                  # W1 quant scale exponent (W1 * 2^13)
DESCALE = 2.0 ** (-(XS + WS))


def build_program(repeat: int = 0, apc: int = APC, b: int = B,
                  gelu_af=AF.Gelu, stage2: str = "dve", defer: int = 2):
    nch = b // 512
    nc = bacc.Bacc("TRN2", target_bir_lowering=False, debug=False)
    x8_d = nc.dram_tensor("x8", [KB, 128, 2, b], F8, kind="ExternalInput").ap()
    w1_d = nc.dram_tensor("w1p", [apc, MT, 128, KB * 256], F8,
                          kind="ExternalInput").ap()
    b1_d = nc.dram_tensor("b1p", [apc, 128, MT], F32, kind="ExternalInput").ap()
    w2_d = nc.dram_tensor("w2p", [apc, 128, MT], F32, kind="ExternalInput").ap()
    b2_d = nc.dram_tensor("b2p", [apc, 1], F32, kind="ExternalInput").ap()
    y_d = nc.dram_tensor("y", [apc, b], F32, kind="ExternalOutput").ap()

    with tile.TileContext(nc) as tc, ExitStack() as ctx:
        const = ctx.enter_context(tc.tile_pool(name="const", bufs=1))
        xp = ctx.enter_context(tc.tile_pool(name="xp", bufs=2))
        wp = ctx.enter_context(tc.tile_pool(name="wp", bufs=3))
        sp = ctx.enter_context(tc.tile_pool(name="sp", bufs=4))
        hp = ctx.enter_context(tc.tile_pool(name="hp", bufs=5))
        ap = ctx.enter_context(tc.tile_pool(name="ap", bufs=2))
        ab = ctx.enter_context(tc.tile_pool(name="ab", bufs=6))
        lg = ctx.enter_context(tc.tile_pool(name="lg", bufs=1))
        ps1 = ctx.enter_context(tc.tile_pool(name="ps1", bufs=3, space="PSUM"))
        ps2 = ctx.enter_context(tc.tile_pool(name="ps2", bufs=2, space="PSUM"))

        def dma_head(a):
            tiles = []
            for m in range(MT):
                t = wp.tile([128, KB, 2, 128], F8, tag=f"w{m}")
                nc.sync.dma_start(t[:], w1_d[a, m])
                tiles.append(t)
            return tiles

        def body():
            b1t = const.tile([128, apc * MT], F32, tag="b1t")
            w2t = const.tile([128, apc * MT], F32, tag="w2t")
            b2t = const.tile([apc, 1], F32, tag="b2t")
            ones = nc.const_aps.tensor(1.0, (128, 1), BF16)
            w2tb = None
            if stage2 == "pe":
                w2tb = const.tile([128, apc * MT], BF16, tag="w2tb")
            for a in range(apc):
                nc.sync.dma_start(b1t[:, a * MT:(a + 1) * MT], b1_d[a])
                nc.sync.dma_start(w2t[:, a * MT:(a + 1) * MT], w2_d[a])
            nc.sync.dma_start(b2t[:], b2_d[:])
            if stage2 == "pe":
                nc.vector.tensor_copy(w2tb[:], w2t[:])

            bh = b // 2           # batch half resident in SBUF at a time
            nchh = bh // 512
            jobs = [(h, a) for h in range(2) for a in range(apc)]

            def dma_xhalf(h):
                tiles = []
                for kb in range(KB):
                    t = xp.tile([128, 2, bh], F8, tag=f"xq{kb}", name="t")
                    nc.sync.dma_start(t[:],
                                      x8_d[kb, :, :, h * bh:(h + 1) * bh])
                    tiles.append(t)
                return tiles

            logits = lg.tile([apc, b], F32, tag="logits")
            yt = lg.tile([apc, b], F32, tag="yt")
            if stage2 in ("none", "act"):
                nc.gpsimd.memset(logits[:], 0.0)
            xqs = {0: dma_xhalf(0)}
            # W1 prefetch two jobs (~110us) ahead so a contended 2 MiB head
            # DMA can never stall a job boundary (wp bufs=3).
            wtiles = [dma_head(jobs[0][1]), dma_head(jobs[1][1])]
            tails = []
            for j, (h, a) in enumerate(jobs):
                wcur = wtiles[j]
                if j + 2 < len(jobs):
                    wtiles.append(dma_head(jobs[j + 2][1]))
                if apc >= 2 and j == apc + 1 and stage2 == "dve":
                    # half-0 epilogue overlapped under half-1 compute: its
                    # tails all flushed by job apc, so sigmoid + y DMA for
                    # the first batch half run on the idle ACT/DMA now
                    # instead of serializing in the end-of-iteration drain.
                    nc.scalar.activation(yt[:, 0:bh], logits[:, 0:bh],
                                         AF.Sigmoid, bias=b2t[:])
                    nc.sync.dma_start(y_d[:, 0:bh], yt[:, 0:bh])
                if j == max(apc - 2, 0) and 1 not in xqs:
                    # half-1 x issued two jobs (~110us) before first use so
                    # the 4.2 MiB transfer never gates the half boundary
                    # even when sharing queues with W1 prefetches
                    xqs[1] = dma_xhalf(1)
                xq = xqs[h]
                flushed = False
                # chunk-PAIR fusion: two 512-batch chunks of the same m-tile
                # share the b1 bias column, so one ACT gelu (and one DVE
                # accumulate op) covers a [128, 1024] region spanning two
                # PSUM banks -- halving ScalarE/VectorE instruction count
                # (ScalarE is the near-critical engine at ~97% busy).
                for pr in range(nchh // 2):
                    acc = None
                    accb = None
                    pending = []
                    for m in range(MT):
                        pt = ps1.tile([128, 1024], F32, tag="ps1")
                        for kb in range(KB):
                            for n01 in range(2):
                                nc.tensor.matmul(
                                    pt[:, n01 * 512:(n01 + 1) * 512],
                                    wcur[m][:, kb],
                                    xq[kb][:, :,
                                           pr * 1024 + n01 * 512:
                                           pr * 1024 + (n01 + 1) * 512],
                                    start=(kb == 0), stop=(kb == KB - 1),
                                    perf_mode=DR, skip_group_check=True)
                        if not flushed and m == defer - 1:
                            # batch-flush the previous job's tails here (one
                            # DR->bf16 mode-switch site per job)
                            while tails:
                                tails.pop(0)()
                            flushed = True
                        if stage2 == "pe" and pending:
                            pending.pop(0)()
                        if stage2 == "none":
                            continue
                        ht = hp.tile([128, 1024], BF16, tag="ht", name="ht")
                        nc.scalar.activation(
                            ht[:], pt[:], gelu_af,
                            bias=b1t[:, a * MT + m:a * MT + m + 1],
                            scale=DESCALE)
                        if stage2 == "act":
                            continue
                        w2c = w2t[:, a * MT + m:a * MT + m + 1]
                        if stage2 == "pe":
                            if m == 0:
                                psy = ps2.tile([1, 1024], F32, tag="psy",
                                               name="psy")

                            def emit_stage2(m=m, ht=ht, psy=psy):
                                nc.tensor.matmul(
                                    psy[:], w2tb[:, a * MT + m:a * MT + m + 1],
                                    ht[:],
                                    start=(m == 0), stop=(m == MT - 1),
                                    skip_group_check=True)
                            pending.append(emit_stage2)
                        elif m == 0:
                            acc = ap.tile([128, 1024], F32, tag="acc",
                                          name="acc")
                            nc.vector.tensor_scalar_mul(acc[:], ht[:], w2c)
                        elif m < MT - 1:
                            nc.vector.scalar_tensor_tensor(
                                acc[:], ht[:], w2c, acc[:],
                                mybir.AluOpType.mult, mybir.AluOpType.add)
                        else:
                            accb = ab.tile([128, 1024], BF16, tag="accb",
                                           name="accb")
                            nc.vector.scalar_tensor_tensor(
                                accb[:], ht[:], w2c, acc[:],
                                mybir.AluOpType.mult, mybir.AluOpType.add)
                    while pending:
                        pending.pop(0)()
                    if stage2 in ("none", "act"):
                        continue

                    off = h * bh + pr * 1024

                    def emit_tail(a=a, off=off, accb=accb):
                        for n01 in range(2):
                            psy = ps2.tile([1, 512], F32, tag="psy",
                                           name="psy")
                            nc.tensor.matmul(
                                psy[:], ones,
                                accb[:, n01 * 512:(n01 + 1) * 512],
                                start=True, stop=True,
                                skip_group_check=True)
                            stg = sp.tile([1, 512], F32, tag="stg",
                                          name="stg")
                            nc.vector.tensor_copy(stg[:], psy[:])
                            nc.sync.dma_start(
                                logits[a:a + 1,
                                       off + n01 * 512:off + (n01 + 1) * 512],
                                stg[:])
                    tails.append(emit_tail)
            while tails:
                tails.pop(0)()
            if apc >= 2 and stage2 == "dve":
                nc.scalar.activation(yt[:, bh:b], logits[:, bh:b],
                                     AF.Sigmoid, bias=b2t[:])
                nc.sync.dma_start(y_d[:, bh:b], yt[:, bh:b])
            else:
                nc.scalar.activation(yt[:], logits[:], AF.Sigmoid,
                                     bias=b2t[:])
                nc.sync.dma_start(y_d[:], yt[:])

        if repeat and repeat > 1:
            with tc.For_i(0, repeat, 1):
                body()
        else:
            body()
    nc.compile()
    return nc


class _Runner:
    """jit-once PJRT runner for a prebuilt Bass program (8-core SPMD)."""

    def __init__(self, nc, n_cores):
        bass2jax.install_neuronx_cc_hook()
        self.nc = nc
        self.n_cores = n_cores
        in_names, out_names, out_avals, zero_outs = [], [], [], []
        for alloc in nc.m.functions[0].allocations:
            if not isinstance(alloc, mybir.MemoryLocationSet):
                continue
            name = alloc.memorylocations[0].name
            if alloc.kind == "ExternalInput":
                in_names.append(name)
            elif alloc.kind == "ExternalOutput":
                shape = tuple(alloc.tensor_shape)
                dtype = mybir.dt.np(alloc.dtype)
                out_names.append(name)
                out_avals.append(jax.core.ShapedArray(shape, dtype))
                zero_outs.append(np.zeros(shape, dtype))
        partition_name = (nc.partition_id_tensor.name
                          if nc.partition_id_tensor else None)
        if partition_name is not None and partition_name in in_names:
            in_names.remove(partition_name)
        self.in_names = in_names
        self.out_names = out_names
        self.zero_outs = zero_outs
        n_params = len(in_names)
        n_outs = len(out_avals)
        all_in_names = list(in_names) + list(out_names)
        if partition_name is not None:
            all_in_names.append(partition_name)
        donate = tuple(range(n_params, n_params + n_outs))

        def _body(*args):
            operands = list(args)
            if partition_name is not None:
                operands.append(bass2jax.partition_id_tensor())
            outs = bass2jax._bass_exec_p.bind(
                *operands,
                out_avals=tuple(out_avals),
                in_names=tuple(all_in_names),
                out_names=tuple(out_names),
                lowering_input_output_aliases=(),
                sim_require_finite=True,
                sim_require_nnan=True,
                nc=nc,
            )
            return tuple(outs)

        devices = jax.devices()[:n_cores]
        assert len(devices) == n_cores, f"need {n_cores} neuron cores"
        self.mesh = Mesh(np.asarray(devices), ("core",))
        in_specs = (PartitionSpec("core"),) * (n_params + n_outs)
        out_specs = (PartitionSpec("core"),) * n_outs
        self.fn = jax.jit(
            shard_map(_body, mesh=self.mesh, in_specs=in_specs,
                      out_specs=out_specs, check_rep=False),
            donate_argnums=donate, keep_unused=True,
        )
        self._dev_inputs = None

    def put_inputs(self, in_maps):
        sharding = NamedSharding(self.mesh, PartitionSpec("core"))
        self._dev_inputs = [
            jax.device_put(
                np.concatenate([np.asarray(m[name]) for m in in_maps], axis=0),
                sharding)
            for name in self.in_names
        ]

    def run(self):
        sharding = NamedSharding(self.mesh, PartitionSpec("core"))
        zouts = [jax.device_put(np.concatenate([z] * self.n_cores, axis=0),
                                sharding) for z in self.zero_outs]
        outs = self.fn(*self._dev_inputs, *zouts)
        jax.block_until_ready(outs)
        return outs

    def run_np(self):
        outs = self.run()
        res = []
        for c in range(self.n_cores):
            d = {}
            for i, name in enumerate(self.out_names):
                full = np.asarray(outs[i])
                per = full.shape[0] // self.n_cores
                d[name] = full[c * per:(c + 1) * per]
            res.append(d)
        return res


_CACHE = {}


def _get_runner(repeat=0):
    if repeat not in _CACHE:
        _CACHE[repeat] = _Runner(build_program(repeat), NCORES)
    return _CACHE[repeat]


def _f8(v, sexp):
    return np.asarray(v * (2.0 ** sexp), dtype=ml_dtypes.float8_e4m3)


def pack_x(x):
    # x8[kb, p, i*B + col] = e4m3(x[col, kb*256 + i*128 + p] * 2^XS)
    b = x.shape[0]
    x8 = _f8(np.asarray(x, dtype=np.float32), XS)        # [B, D]
    x8 = np.ascontiguousarray(
        x8.T.reshape(KB, 2, 128, b).transpose(0, 2, 1, 3))
    return x8


def pack_w1(W1):
    # w18[a, m, p, kb*256 + i*128 + c] = e4m3(W1[a, kb*256+i*128+p, m*128+c]
    #                                          * 2^WS)
    a = W1.shape[0]
    w8 = _f8(np.asarray(W1, dtype=np.float32), WS)       # [A, D, H]
    w8 = np.ascontiguousarray(
        w8.reshape(a, KB, 2, 128, MT, 128).transpose(0, 4, 3, 1, 2, 5)
        .reshape(a, MT, 128, KB * 256))
    return w8


def make_in_maps(x, W1, b1, W2, b2):
    b1 = np.asarray(b1, dtype=np.float32)
    W2 = np.asarray(W2, dtype=np.float32)
    b2 = np.asarray(b2, dtype=np.float32)
    x8 = pack_x(x)
    W1p = pack_w1(W1)
    b1p = np.ascontiguousarray(b1.reshape(A, MT, 128).transpose(0, 2, 1))
    W2p = np.ascontiguousarray(W2.reshape(A, MT, 128).transpose(0, 2, 1))
    b2p = np.ascontiguousarray(b2.reshape(A, 1))
    in_maps = []
    for c in range(NCORES):
        s = slice(c * APC, (c + 1) * APC)
        in_maps.append({"x8": x8, "w1p": W1p[s], "b1p": b1p[s],
                        "w2p": W2p[s], "b2p": b2p[s]})
    return in_maps


def kernel(x, W1, b1, W2, b2):
    in_maps = make_in_maps(x, W1, b1, W2, b2)
    r = _get_runner(0)
    r.put_inputs(in_maps)
    outs = r.run_np()
    y = np.concatenate([outs[c]["y"] for c in range(NCORES)], axis=0)
    return np.ascontiguousarray(y.T).astype(np.float32)
